# revision 4
# baseline (speedup 1.0000x reference)
"""Trainium2 Bass kernel: y = x @ (int8_w * scale_per_col).

Strategy (column-parallel over 8 NeuronCores):
  - core i owns 2304 output columns; x is replicated.
  - int8 weights are exactly representable in bf16.  x (fp32) is split on the
    host into hi/lo bf16 halves (x = hi + lo to ~16 mantissa bits), and the
    device does two accumulating bf16 matmul passes into fp32 PSUM, so the
    result matches the fp32 reference to ~1e-5 without paying the 4x cost of
    native fp32 matmul.
  - the per-output-channel scale commutes with the contraction, so it is
    applied by the vector engine while evicting PSUM -> SBUF.
  - all operands are pre-tiled on the host into PE-native [partition, free]
    layouts so every DMA is a fully contiguous per-partition stream.

Per-core geometry: M=4096 (32 m-tiles), K=7168 (56 k-tiles), Nshard=2304
(3 column groups x 768; each group = 2 PSUM chunks of 384).  The w panel for
one column group (11MB) stays resident in SBUF while the m-loop streams x
tiles, so x traffic is 3 sweeps (352MB/core) and w traffic is 1 sweep.
"""

import os
import sys

sys.path.insert(0, "/opt/trn_rl_repo")

import numpy as np
import ml_dtypes

BF16 = ml_dtypes.bfloat16

# full problem
M, K, N = 4096, 7168, 18432
NCORES = 8
NSH = N // NCORES  # 2304 columns per core

# per-core tiling
P = 128
MT = M // P            # 32 m-tiles
KT = K // P            # 56 k-tiles
KSUB = 7               # k-tiles per w DMA group
KGRP = KT // KSUB      # 8 w groups per column-group panel
NGW = 768              # columns per w panel (column group)
CG = NSH // NGW        # 3 column groups
NCH = 384              # columns per matmul / PSUM chunk
NCPG = NGW // NCH      # 2 chunks per column group

_LAST = {"exec_time_ns": None, "results": None}


def _prep_x(x):
    """fp32 [M, K] -> (hi, lo) bf16 tiles [MT, P(k), KT, P(m)]."""
    hi = x.astype(BF16)
    lo = (x - hi.astype(np.float32)).astype(BF16)

    def tilt(a):
        # a[mt*128+am, kt*128+bk] -> out[mt, bk, kt, am]
        return np.ascontiguousarray(
            a.reshape(a.shape[0] // P, P, a.shape[1] // P, P)
            .transpose(0, 3, 2, 1)
        )

    return tilt(hi), tilt(lo)


def _prep_w_core(weight, scale, i):
    """Core i's weight shard -> bf16 tiles [CG, KGRP, P(k), KSUB, NGW]."""
    wc = weight[:, i * NSH : (i + 1) * NSH].astype(BF16)  # int8 exact in bf16
    # wc[g*KSUB*128 + kt*128 + p, cg*NGW + nn] -> [cg, g, p, kt, nn]
    wt = np.ascontiguousarray(
        wc.reshape(KGRP, KSUB, P, CG, NGW).transpose(3, 0, 2, 1, 4)
    )
    sb = np.ascontiguousarray(
        np.broadcast_to(
            scale[i * NSH : (i + 1) * NSH].astype(np.float32), (P, NSH)
        )
    )
    return wt, sb


def _build_program(mt, cg, kgrp):
    import concourse.mybir as mybir
    import concourse.tile as tile
    from concourse import bacc

    bf = mybir.dt.bfloat16
    f32 = mybir.dt.float32
    kt = kgrp * KSUB
    nsh = cg * NGW

    nc = bacc.Bacc("TRN2", target_bir_lowering=False, debug=False,
                   num_devices=NCORES)
    xh_d = nc.dram_tensor("xt_hi", [mt, P, kt, P], bf, kind="ExternalInput").ap()
    xl_d = nc.dram_tensor("xt_lo", [mt, P, kt, P], bf, kind="ExternalInput").ap()
    w_d = nc.dram_tensor("w_t", [cg, kgrp, P, KSUB, NGW], bf,
                         kind="ExternalInput").ap()
    s_d = nc.dram_tensor("scale_b", [P, nsh], f32, kind="ExternalInput").ap()
    y_d = nc.dram_tensor("y", [mt * P, nsh], f32, kind="ExternalOutput").ap()

    with tile.TileContext(nc) as tc:
        with tc.tile_pool(name="wp", bufs=min(kgrp + 2, 10)) as wp, \
             tc.tile_pool(name="xp", bufs=4) as xp, \
             tc.tile_pool(name="sp", bufs=1) as sp, \
             tc.tile_pool(name="op", bufs=4) as op, \
             tc.tile_pool(name="pp", bufs=4, space="PSUM") as pp:
            scale_sb = sp.tile([P, nsh], f32, tag="scale", name="scale_sb")
            nc.sync.dma_start(out=scale_sb, in_=s_d)
            for c_i in range(cg):
                wts = []
                for g in range(kgrp):
                    wt_t = wp.tile([P, KSUB, NGW], bf, tag="w",
                                   name=f"w_{c_i}_{g}")
                    nc.sync.dma_start(out=wt_t, in_=w_d[c_i, g])
                    wts.append(wt_t)
                for m_i in range(mt):
                    xh = xp.tile([P, kt, P], bf, tag="x", name=f"xh_{c_i}_{m_i}")
                    nc.sync.dma_start(out=xh, in_=xh_d[m_i])
                    xl = xp.tile([P, kt, P], bf, tag="x", name=f"xl_{c_i}_{m_i}")
                    nc.sync.dma_start(out=xl, in_=xl_d[m_i])
                    ps = [
                        pp.tile([P, NCH], f32, tag="ps",
                                name=f"ps_{c_i}_{m_i}_{c}")
                        for c in range(NCPG)
                    ]
                    for k in range(kt):
                        g, kk = divmod(k, KSUB)
                        for is_hi, xt_t in ((True, xh), (False, xl)):
                            for c in range(NCPG):
                                nc.tensor.matmul(
                                    ps[c],
                                    xt_t[:, k],
                                    wts[g][:, kk, c * NCH : (c + 1) * NCH],
                                    start=(k == 0 and is_hi),
                                    stop=(k == kt - 1 and not is_hi),
                                )
                    for c in range(NCPG):
                        ot = op.tile([P, NCH], f32, tag="o",
                                     name=f"o_{c_i}_{m_i}_{c}")
                        n0 = c_i * NGW + c * NCH
                        nc.vector.tensor_mul(
                            out=ot, in0=ps[c],
                            in1=scale_sb[:, n0 : n0 + NCH],
                        )
                        nc.sync.dma_start(
                            out=y_d[m_i * P : (m_i + 1) * P, n0 : n0 + NCH],
                            in_=ot,
                        )
    nc.compile()
    return nc


def _run(xh_t, xl_t, w_ts, s_bs, mt, cg, kgrp, trace=False):
    from concourse.bass_utils import run_bass_kernel_spmd

    nc = _build_program(mt, cg, kgrp)
    in_maps = [
        {"xt_hi": xh_t, "xt_lo": xl_t, "w_t": w_ts[i], "scale_b": s_bs[i]}
        for i in range(NCORES)
    ]
    res = run_bass_kernel_spmd(nc, in_maps, list(range(NCORES)), trace=trace)
    _LAST["exec_time_ns"] = res.exec_time_ns
    return [np.asarray(r["y"], dtype=np.float32) for r in res.results]


def kernel(x, weight, scale, _trace=False):
    x = np.ascontiguousarray(np.asarray(x, dtype=np.float32))
    weight = np.ascontiguousarray(np.asarray(weight, dtype=np.int8))
    scale = np.asarray(scale, dtype=np.float32)

    from concurrent.futures import ThreadPoolExecutor

    with ThreadPoolExecutor(max_workers=8) as ex:
        fx = ex.submit(_prep_x, x)
        fw = [ex.submit(_prep_w_core, weight, scale, i) for i in range(NCORES)]
        xh_t, xl_t = fx.result()
        per_core = [f.result() for f in fw]
    w_ts = [pc[0] for pc in per_core]
    s_bs = [pc[1] for pc in per_core]

    trace = _trace or bool(os.environ.get("KERNEL_TRACE"))
    shards = _run(xh_t, xl_t, w_ts, s_bs, MT, CG, KGRP, trace=trace)
    return np.concatenate(shards, axis=1)
